# revision 19
# baseline (speedup 1.0000x reference)
"""DiffGCN Trainium2 kernel: 8-core SPMD, node-sharded walks.

Matches reference.py of nn_DiffGCN_46351287058748:
  - T=4 diffusion steps over N=50000 nodes, degree D=16, C=128 channels.
  - Per step, each walk scores its 16 candidate neighbours with a 2-layer MLP
    whose first-layer contribution per candidate is a table lookup
    U_t[n] = node_attr[n] @ W1_block(t+1) (|W2|-scaled, sign-permuted cols),
    plus a per-walk running prefix h_pre; relu; signed reduce -> logp;
    softmax + noise + argmax picks the next node.
  - A GRU (torch gate order r,z,n) runs over the 5 walk embeddings; out @ Wo.

Sharding: walks (rows) are split across 8 cores, 6250 each (padded to 6272).
Every core keeps a full replicated copy of node_attr / adjacency / U tables
in its own DRAM - no inter-core communication.

Gathers use the int16 dma_gather ucode. Node ids exceed int16 range, so all
tables are addressed by 512B+ "pair rows" (idx = id>>1 <= 25599) and the
correct half is selected on-device with a parity mask. The U table uses an
interleaved row order o(n) (so the U-phase writes stream out as contiguous
1KB descriptors); the adjacency table stores o(dst[e]) pre-transformed on
the host, and the true node id is recovered algebraically after argmax.
"""

import numpy as np

import concourse.bacc as bacc
import concourse.bass as bass
import concourse.mybir as mybir
import concourse.tile as tile
from concourse import bass_utils
from concourse.masks import make_identity

F32 = mybir.dt.float32
I32 = mybir.dt.int32
I16 = mybir.dt.int16
AF = mybir.ActivationFunctionType
ALU = mybir.AluOpType
AX = mybir.AxisListType

P = 128
N = 50000
C = 128
D = 16
T = 4
HM = 64            # diff_mlp hidden
HG = 128           # GRU hidden
NCORES = 8
WPC = N // NCORES              # 6250 walks per core
NT = 49                        # walk tiles -> 6272 padded walks
WPAD = NT * P
SUP = 2                        # walk tiles per main gather (4096 idxs)
NPAD = 51200                   # node rows padded to 25 * 2048
NB = NPAD // 2048              # U-phase rounds (25)
ECH = 13                       # walk-tiles per emb gather call

_CACHE = {}
TRACE = False          # test harness can flip this to get an NTFF profile
LAST_EXEC_NS = None
LAST_RESULTS = None


def _stripe_shuffle(nc, dst16, src32, ncols):
    """Build the dma_gather int16 index stream.

    Index k of the stream lives at dst16[k%16, k//16] (replicated over the 8
    16-partition stripes).  Stream position r = q*128+p must hold
    src32[p, q] (low 16 bits), so dst16[t, q*8+u] = lo16(src32[u*16+t, q]).
    """
    src16 = src32.bitcast(I16)
    for u in range(8):
        nc.sync.dma_start(
            out=dst16[0:16, 0:ncols * 8].rearrange("p (q u) -> p q u", u=8)[:, :, u],
            in_=src16[u * 16:(u + 1) * 16, 0:2 * ncols]
            .rearrange("p (q h) -> p q h", h=2)[:, :, 0],
        )
    for s in range(1, 8):
        nc.sync.dma_start(out=dst16[s * 16:(s + 1) * 16, 0:ncols * 8],
                          in_=dst16[0:16, 0:ncols * 8])


def _build(k_pos: int):
    nc = bacc.Bacc("TRN2", target_bir_lowering=False, debug=False,
                   num_devices=NCORES)

    natT = nc.dram_tensor("natT", [P, NPAD], F32, kind="ExternalInput")
    nat2 = nc.dram_tensor("nat2", [NPAD // 2, 2 * C], F32, kind="ExternalInput")
    natTo = nc.dram_tensor("natTo", [P, WPAD], F32, kind="ExternalInput")
    quad = nc.dram_tensor("quad", [NPAD // 4, 4 * D], I32, kind="ExternalInput")
    cur0 = nc.dram_tensor("cur0", [P, NT], I32, kind="ExternalInput")
    noiseR = nc.dram_tensor("noiseR", [T, P, NT * D], F32, kind="ExternalInput")
    w1u = nc.dram_tensor("w1u", [P, T * HM], F32, kind="ExternalInput")
    w10 = nc.dram_tensor("w10", [P, HM], F32, kind="ExternalInput")
    b1r = nc.dram_tensor("b1r", [P, HM], F32, kind="ExternalInput")
    wxd = nc.dram_tensor("wxd", [P, 3 * HG], F32, kind="ExternalInput")
    whd = nc.dram_tensor("whd", [P, 3 * HG], F32, kind="ExternalInput")
    bgx = nc.dram_tensor("bgx", [P, 3], F32, kind="ExternalInput")
    bgh = nc.dram_tensor("bgh", [P, 3], F32, kind="ExternalInput")
    bsum = nc.dram_tensor("bsum", [P, 4], F32, kind="ExternalInput")
    wod = nc.dram_tensor("wod", [P, C], F32, kind="ExternalInput")
    bord = nc.dram_tensor("bord", [P, C], F32, kind="ExternalInput")
    outT = nc.dram_tensor("outT", [WPAD, C], F32, kind="ExternalOutput")

    utab = [nc.dram_tensor(f"utab{t}", [NPAD * HM], F32, kind="Internal")
            for t in range(T)]
    xtd = nc.dram_tensor("xtd", [T, P, WPAD], F32, kind="Internal")

    with tile.TileContext(nc) as tc:
        with (
            tc.tile_pool(name="const", bufs=1) as cp,
            tc.tile_pool(name="state", bufs=1) as st,
            tc.tile_pool(name="stg", bufs=4) as sg,
        ):
            w1u_s = cp.tile([P, T * HM], F32)
            nc.sync.dma_start(out=w1u_s[:], in_=w1u.ap())
            w10_s = cp.tile([P, HM], F32)
            nc.sync.dma_start(out=w10_s[:], in_=w10.ap())
            b1r_s = cp.tile([P, HM], F32)
            nc.sync.dma_start(out=b1r_s[:], in_=b1r.ap())
            bgx_s = cp.tile([P, 3], F32)
            nc.sync.dma_start(out=bgx_s[:], in_=bgx.ap())
            bgh_s = cp.tile([P, 3], F32)
            nc.sync.dma_start(out=bgh_s[:], in_=bgh.ap())
            bsum_s = cp.tile([P, 4], F32)
            nc.sync.dma_start(out=bsum_s[:], in_=bsum.ap())
            wx_s = cp.tile([P, 3 * HG], F32)
            nc.sync.dma_start(out=wx_s[:], in_=wxd.ap())
            wh_s = cp.tile([P, 3 * HG], F32)
            nc.sync.dma_start(out=wh_s[:], in_=whd.ap())
            wo_s = cp.tile([P, C], F32)
            nc.sync.dma_start(out=wo_s[:], in_=wod.ap())
            bor_s = cp.tile([P, C], F32)
            nc.sync.dma_start(out=bor_s[:], in_=bord.ap())
            ident = cp.tile([P, P], F32)
            make_identity(nc, ident[:])

            # ---------------- U phase ----------------
            # utab[t] flat layout: ((b, p, m, c)) with node n = b*2048+m*128+p
            # at flat offset b*131072 + p*1024 + m*64 + c.
            with (
                tc.tile_pool(name="uph", bufs=2) as up,
                tc.tile_pool(name="upsum", bufs=4, space="PSUM") as ups,
            ):
                for b in range(NB):
                    natc = up.tile([P, 2048], F32, tag="natc")
                    nc.sync.dma_start(out=natc[:],
                                      in_=natT.ap()[:, b * 2048:(b + 1) * 2048])
                    stage = up.tile([P, 16 * T * HM], F32, tag="stage")
                    for m in range(16):
                        ps = ups.tile([P, T * HM], F32, tag="ups")
                        nc.tensor.matmul(ps[:], lhsT=natc[:, m * P:(m + 1) * P],
                                         rhs=w1u_s[:], start=True, stop=True)
                        nc.scalar.copy(
                            out=stage[:, m * T * HM:(m + 1) * T * HM], in_=ps[:])
                    st4 = stage[:].rearrange("p (m t c) -> p m t c", t=T, c=HM)
                    for t in range(T):
                        nc.sync.dma_start(
                            out=utab[t].ap()
                            .rearrange("(b p m c) -> b p m c", p=P, m=16, c=HM)[b],
                            in_=st4[:, :, t, :])

            # ---------------- persistent walk state ----------------
            big = st.tile([P, WPAD], F32)           # natTo / emb staging / GRU h
            nc.sync.dma_start(out=big[:], in_=natTo.ap())
            hpre = st.tile([P, NT * HM], F32)
            curI = st.tile([P, NT], I32)
            nc.sync.dma_start(out=curI[:], in_=cur0.ap())
            curS = st.tile([P, NT], I32)            # scratch for id algebra
            curS2 = st.tile([P, NT], I32)
            mq = st.tile([P, NT], I32)
            mhF = st.tile([P, NT], F32)
            idxQ = st.tile([P, NT * 8], I16)
            idxH = st.tile([P, NT * 8], I16)
            idxU = st.tile([P, NT * D * 8], I16)
            nbrO = st.tile([P, NT * D], I32)        # o-coded candidate ids
            nbrI = st.tile([P, NT * D], I32)        # true candidate ids
            nbrF = st.tile([P, NT * D], F32)
            nbrS = st.tile([P, NT * D], I32)
            parI = st.tile([P, NT * D], I32)
            noiseT = st.tile([P, NT * D], F32)
            logpP = st.tile([P, NT * D], F32)
            logpN = st.tile([P, NT * D], F32)
            qB = st.tile([P, NT * D], F32)
            scr = st.tile([P, NT * D], F32)
            maskB = st.tile([P, NT * D], F32)
            maskI = st.tile([P, NT * D], I32)
            iotaF = st.tile([P, NT * D], F32)
            m49 = st.tile([P, NT], F32)
            s49 = st.tile([P, NT], F32)
            lns = st.tile([P, NT], F32)
            qm49 = st.tile([P, NT], F32)
            selF = st.tile([P, NT], F32)
            curFn = st.tile([P, NT], F32)

            nc.gpsimd.iota(iotaF[:], pattern=[[0, NT], [1, D]], base=0,
                           channel_multiplier=0,
                           allow_small_or_imprecise_dtypes=True)

            # h_pre init: b1 + node_attr_own @ W1s[0:128]
            with tc.tile_pool(name="hpi", bufs=2, space="PSUM") as hpp:
                for g in range(7):
                    lo, hi = g * 8, min(g * 8 + 8, NT)
                    w = hi - lo
                    psd = hpp.tile([P, 8 * HM], F32, tag="hd")
                    for i in range(lo, hi):
                        nc.tensor.matmul(psd[:, (i - lo) * HM:(i - lo + 1) * HM],
                                         lhsT=big[:, i * P:(i + 1) * P],
                                         rhs=w10_s[:], start=True, stop=True)
                    b1b = b1r_s[:].unsqueeze(1).to_broadcast([P, w, HM])
                    nc.vector.scalar_tensor_tensor(
                        out=hpre[:, lo * HM:hi * HM]
                        .rearrange("p (q c) -> p q c", c=HM),
                        in0=psd[:, :w * HM].rearrange("p (q c) -> p q c", c=HM),
                        scalar=0.0, in1=b1b, op0=ALU.bypass, op1=ALU.add)

            # ---------------- diffusion ----------------
            with (
                tc.tile_pool(name="dif", bufs=2) as dp,
                tc.tile_pool(name="difs", bufs=1) as ds,
                tc.tile_pool(name="dpsA", bufs=2, space="PSUM") as psA,
                tc.tile_pool(name="dpsB", bufs=3, space="PSUM") as psB,
            ):
                for t in range(T):
                    # --- candidate ids: o-coded nbr = quad[cur>>2] selected ---
                    nc.vector.tensor_scalar(out=curS[:], in0=curI[:], scalar1=2,
                                            scalar2=None,
                                            op0=ALU.arith_shift_right)
                    _stripe_shuffle(nc, idxQ, curS[:], NT)
                    rawN = ds.tile([P, NT * 4 * D], I32, tag="rawN")
                    nc.gpsimd.dma_gather(
                        out_ap=rawN[:].rearrange("p (q e) -> p q e", e=4 * D),
                        in_ap=quad.ap(), idxs_ap=idxQ[:], num_idxs=NT * P,
                        num_idxs_reg=NT * P, elem_size=4 * D,
                        single_packet=False)
                    nc.vector.tensor_scalar(out=mq[:], in0=curI[:], scalar1=3,
                                            scalar2=None, op0=ALU.bitwise_and)
                    r3 = rawN[:].rearrange("p (q e) -> p q e", e=4 * D)
                    n3 = nbrO[:].rearrange("p (q j) -> p q j", j=D)
                    nc.vector.tensor_copy(out=n3, in_=r3[:, :, 0:D])
                    for cc in range(1, 4):
                        mk = ds.tile([P, NT], I32, tag="mk")
                        nc.vector.tensor_scalar(out=mk[:], in0=mq[:], scalar1=cc,
                                                scalar2=None, op0=ALU.is_equal)
                        tq = ds.tile([P, NT * D], I32, tag="tq")
                        t3 = tq[:].rearrange("p (q j) -> p q j", j=D)
                        nc.vector.tensor_sub(out=t3,
                                             in0=r3[:, :, cc * D:(cc + 1) * D],
                                             in1=n3)
                        nc.vector.tensor_mul(
                            out=t3, in0=t3,
                            in1=mk[:].unsqueeze(2).to_broadcast([P, NT, D]))
                        nc.vector.tensor_add(out=n3, in0=n3, in1=t3)

                    # --- true ids + gather helpers from o-code ---
                    # n = (o & ~2047) | ((o & 15) << 7) | ((o >> 4) & 127)
                    nc.vector.tensor_scalar(out=nbrS[:], in0=nbrO[:],
                                            scalar1=15, scalar2=7,
                                            op0=ALU.bitwise_and,
                                            op1=ALU.logical_shift_left)
                    nc.vector.tensor_scalar(out=nbrI[:], in0=nbrO[:],
                                            scalar1=4, scalar2=127,
                                            op0=ALU.logical_shift_right,
                                            op1=ALU.bitwise_and)
                    nc.vector.tensor_tensor(out=nbrI[:], in0=nbrI[:],
                                            in1=nbrS[:], op=ALU.bitwise_or)
                    nc.vector.tensor_scalar(out=nbrS[:], in0=nbrO[:],
                                            scalar1=-2048, scalar2=None,
                                            op0=ALU.bitwise_and)
                    nc.vector.tensor_tensor(out=nbrI[:], in0=nbrI[:],
                                            in1=nbrS[:], op=ALU.bitwise_or)
                    nc.vector.tensor_copy(out=nbrF[:], in_=nbrI[:])
                    # u-table pair idx / parity from the o-code
                    nc.vector.tensor_scalar(out=nbrS[:], in0=nbrO[:], scalar1=1,
                                            scalar2=None,
                                            op0=ALU.arith_shift_right)
                    _stripe_shuffle(nc, idxU, nbrS[:], NT * D)
                    nc.vector.tensor_scalar(out=parI[:], in0=nbrO[:], scalar1=1,
                                            scalar2=None, op0=ALU.bitwise_and)
                    nc.sync.dma_start(out=noiseT[:], in_=noiseR.ap()[t])

                    # --- score candidates per super-tile ---
                    nsup = (NT + SUP - 1) // SUP
                    for s in range(nsup):
                        lo, hi = s * SUP, min(s * SUP + SUP, NT)
                        w = hi - lo
                        raw = dp.tile([P, SUP * D * 2 * HM], F32, tag="raw")
                        rv = raw[:, :w * D * 2 * HM]
                        nc.gpsimd.dma_gather(
                            out_ap=rv.rearrange("p (q e) -> p q e", e=2 * HM),
                            in_ap=utab[t].ap().rearrange("(r c) -> r c", c=2 * HM),
                            idxs_ap=idxU[:, lo * D * 8:hi * D * 8],
                            num_idxs=w * D * P, num_idxs_reg=w * D * P,
                            elem_size=2 * HM, single_packet=False)
                        # hn rows are padded to HM+1 so the 3-D views below
                        # stay unmergeable (copy_predicated needs congruent
                        # operand shapes in CoreSim).
                        HP = HM + 1
                        hn = dp.tile([P, SUP * D * HP], F32, tag="hn")
                        r3d = rv.rearrange("p (q c) -> p q c", c=2 * HM)
                        h3d = (hn[:, :w * D * HP]
                               .rearrange("p (q c) -> p q c", c=HP)[:, :, 0:HM])
                        nc.scalar.copy(out=h3d, in_=r3d[:, :, 0:HM])
                        pb = (parI[:, lo * D:hi * D].unsqueeze(2)
                              .to_broadcast([P, w * D, HM]))
                        nc.vector.copy_predicated(out=h3d, mask=pb,
                                                  data=r3d[:, :, HM:2 * HM])
                        for q in range(w):
                            hq = (hn[:, q * D * HP:(q + 1) * D * HP]
                                  .rearrange("p (j c) -> p j c", c=HP)[:, :, 0:HM])
                            hp_b = (hpre[:, (lo + q) * HM:(lo + q + 1) * HM]
                                    .unsqueeze(1).to_broadcast([P, D, HM]))
                            nc.vector.scalar_tensor_tensor(
                                out=hq, in0=hq, scalar=0.0, in1=hp_b,
                                op0=ALU.bypass, op1=ALU.add)
                        nc.scalar.activation(out=h3d, in_=h3d, func=AF.Relu)
                        if k_pos > 0:
                            nc.vector.tensor_reduce(
                                out=logpP[:, lo * D:hi * D],
                                in_=h3d[:, :, 0:k_pos], axis=AX.X, op=ALU.add)
                        else:
                            nc.vector.memset(logpP[:, lo * D:hi * D], 0.0)
                        if k_pos < HM:
                            nc.vector.tensor_reduce(
                                out=logpN[:, lo * D:hi * D],
                                in_=h3d[:, :, k_pos:HM], axis=AX.X, op=ALU.add)
                        else:
                            nc.vector.memset(logpN[:, lo * D:hi * D], 0.0)

                    # --- softmax + noise + argmax (batched) ---
                    nc.vector.tensor_sub(out=qB[:], in0=logpP[:], in1=logpN[:])
                    q3 = qB[:].rearrange("p (i j) -> p i j", j=D)
                    nc.vector.tensor_reduce(out=m49[:], in_=q3, axis=AX.X,
                                            op=ALU.max)
                    m_b = m49[:].unsqueeze(2).to_broadcast([P, NT, D])
                    nc.vector.tensor_tensor(out=q3, in0=q3, in1=m_b,
                                            op=ALU.subtract)
                    nc.scalar.activation(out=scr[:], in_=qB[:], func=AF.Exp)
                    s3 = scr[:].rearrange("p (i j) -> p i j", j=D)
                    nc.vector.tensor_reduce(out=s49[:], in_=s3, axis=AX.X,
                                            op=ALU.add)
                    nc.scalar.activation(out=lns[:], in_=s49[:], func=AF.Ln)
                    l_b = lns[:].unsqueeze(2).to_broadcast([P, NT, D])
                    nc.vector.tensor_tensor(out=q3, in0=q3, in1=l_b,
                                            op=ALU.subtract)
                    nc.scalar.activation(out=qB[:], in_=qB[:], func=AF.Exp)
                    nc.vector.tensor_add(out=qB[:], in0=qB[:], in1=noiseT[:])
                    nc.vector.tensor_reduce(out=qm49[:], in_=q3, axis=AX.X,
                                            op=ALU.max)
                    qm_b = qm49[:].unsqueeze(2).to_broadcast([P, NT, D])
                    mi3 = maskI[:].rearrange("p (i j) -> p i j", j=D)
                    nc.vector.tensor_tensor(out=mi3, in0=q3, in1=qm_b,
                                            op=ALU.is_equal)
                    nc.vector.memset(scr[:], 1.0e9)
                    nc.vector.copy_predicated(out=scr[:], mask=maskI[:],
                                              data=iotaF[:])
                    sc3 = scr[:].rearrange("p (i j) -> p i j", j=D)
                    nc.vector.tensor_reduce(out=selF[:], in_=sc3, axis=AX.X,
                                            op=ALU.min)
                    s_b = selF[:].unsqueeze(2).to_broadcast([P, NT, D])
                    i3 = iotaF[:].rearrange("p (i j) -> p i j", j=D)
                    nc.vector.tensor_tensor(out=maskB[:], in0=i3, in1=s_b,
                                            op=ALU.is_equal)
                    nc.vector.tensor_mul(out=maskB[:], in0=maskB[:], in1=nbrF[:])
                    mk3 = maskB[:].rearrange("p (i j) -> p i j", j=D)
                    nc.vector.tensor_reduce(out=curFn[:], in_=mk3, axis=AX.X,
                                            op=ALU.add)
                    nc.vector.tensor_copy(out=curI[:], in_=curFn[:])

                    # --- walk embeddings for the chosen nodes ---
                    nc.vector.tensor_scalar(out=curS[:], in0=curI[:], scalar1=1,
                                            scalar2=None,
                                            op0=ALU.arith_shift_right)
                    _stripe_shuffle(nc, idxH, curS[:], NT)
                    nc.vector.tensor_scalar(out=curS2[:], in0=curI[:], scalar1=1,
                                            scalar2=None, op0=ALU.bitwise_and)
                    nc.vector.tensor_copy(out=mhF[:], in_=curS2[:])
                    e3 = big[:].rearrange("p (i c) -> p i c", c=C)
                    for ec in range((NT + ECH - 1) // ECH):
                        lo, hi = ec * ECH, min(ec * ECH + ECH, NT)
                        w = hi - lo
                        rawE = dp.tile([P, ECH * 2 * C], F32, tag="rawE")
                        rev = rawE[:, :w * 2 * C]
                        nc.gpsimd.dma_gather(
                            out_ap=rev.rearrange("p (q e) -> p q e", e=2 * C),
                            in_ap=nat2.ap(),
                            idxs_ap=idxH[:, lo * 8:hi * 8],
                            num_idxs=w * P, num_idxs_reg=w * P,
                            elem_size=2 * C, single_packet=False)
                        re3 = rev.rearrange("p (q c) -> p q c", c=2 * C)
                        te = ds.tile([P, ECH * C], F32, tag="te")
                        t3e = te[:, :w * C].rearrange("p (q c) -> p q c", c=C)
                        nc.vector.tensor_sub(out=t3e, in0=re3[:, :, C:2 * C],
                                             in1=re3[:, :, 0:C])
                        mb = (mhF[:, lo:hi].unsqueeze(2)
                              .to_broadcast([P, w, C]))
                        nc.vector.tensor_mul(out=t3e, in0=t3e, in1=mb)
                        nc.vector.tensor_add(out=e3[:, lo:hi, :],
                                             in0=re3[:, :, 0:C], in1=t3e)

                    # --- transpose, store xT, h_pre += emb @ W1s[t+1] ---
                    for g in range(7):
                        lo, hi = g * 8, min(g * 8 + 8, NT)
                        w = hi - lo
                        psd = psA.tile([P, 8 * HM], F32, tag="hd")
                        for i in range(lo, hi):
                            pst = psB.tile([P, P], F32, tag="tp")
                            nc.tensor.transpose(pst[:], big[:, i * P:(i + 1) * P],
                                                ident[:])
                            stg = sg.tile([P, P], F32, tag="stg")
                            nc.scalar.copy(out=stg[:], in_=pst[:])
                            nc.sync.dma_start(
                                out=xtd.ap()[t, :, i * P:(i + 1) * P],
                                in_=stg[:])
                            nc.tensor.matmul(
                                psd[:, (i - lo) * HM:(i - lo + 1) * HM],
                                lhsT=stg[:], rhs=w1u_s[:, t * HM:(t + 1) * HM],
                                start=True, stop=True)
                        nc.vector.tensor_add(out=hpre[:, lo * HM:hi * HM],
                                             in0=hpre[:, lo * HM:hi * HM],
                                             in1=psd[:, :w * HM])

            # ---------------- GRU ----------------
            hT = big                                  # reuse as h state
            CHW = 512
            nch = (WPAD + CHW - 1) // CHW
            with (
                tc.tile_pool(name="gru", bufs=3) as gp,
                tc.tile_pool(name="gpsum", bufs=2, space="PSUM") as gps,
            ):
                for step in range(T + 1):
                    first = step == 0
                    for ci in range(nch):
                        c0 = ci * CHW
                        c1 = min(c0 + CHW, WPAD)
                        w = c1 - c0
                        xc = gp.tile([P, CHW], F32, tag="xc")
                        if first:
                            nc.sync.dma_start(out=xc[:, :w],
                                              in_=natTo.ap()[:, c0:c1])
                        else:
                            nc.sync.dma_start(out=xc[:, :w],
                                              in_=xtd.ap()[step - 1, :, c0:c1])
                        psr = gps.tile([P, CHW], F32, tag="gr")
                        psz = gps.tile([P, CHW], F32, tag="gz")
                        psn = gps.tile([P, CHW], F32, tag="gn")
                        nc.tensor.matmul(psr[:, :w], lhsT=wx_s[:, 0:HG],
                                         rhs=xc[:, :w], start=True, stop=first)
                        nc.tensor.matmul(psz[:, :w], lhsT=wx_s[:, HG:2 * HG],
                                         rhs=xc[:, :w], start=True, stop=first)
                        nc.tensor.matmul(psn[:, :w], lhsT=wx_s[:, 2 * HG:3 * HG],
                                         rhs=xc[:, :w], start=True, stop=True)
                        if not first:
                            psh = gps.tile([P, CHW], F32, tag="gh")
                            hsl = hT[:, c0:c1]
                            nc.tensor.matmul(psr[:, :w], lhsT=wh_s[:, 0:HG],
                                             rhs=hsl, start=False, stop=True)
                            nc.tensor.matmul(psz[:, :w], lhsT=wh_s[:, HG:2 * HG],
                                             rhs=hsl, start=False, stop=True)
                            nc.tensor.matmul(psh[:, :w], lhsT=wh_s[:, 2 * HG:],
                                             rhs=hsl, start=True, stop=True)
                        rg = gp.tile([P, CHW], F32, tag="rg")
                        nc.scalar.activation(out=rg[:, :w], in_=psr[:, :w],
                                             func=AF.Sigmoid,
                                             bias=bsum_s[:, 0:1])
                        t1 = gp.tile([P, CHW], F32, tag="t1")
                        if first:
                            # hn-gate = bgh_n only: t1 = r * bgh_n
                            nc.vector.tensor_scalar(
                                out=t1[:, :w], in0=rg[:, :w],
                                scalar1=bgh_s[:, 2:3], scalar2=None,
                                op0=ALU.mult)
                        else:
                            nc.vector.scalar_tensor_tensor(
                                out=t1[:, :w], in0=psh[:, :w],
                                scalar=bgh_s[:, 2:3], in1=rg[:, :w],
                                op0=ALU.add, op1=ALU.mult)
                        t2 = gp.tile([P, CHW], F32, tag="t2")
                        nc.vector.scalar_tensor_tensor(
                            out=t2[:, :w], in0=psn[:, :w],
                            scalar=bgx_s[:, 2:3], in1=t1[:, :w],
                            op0=ALU.add, op1=ALU.add)
                        ng = gp.tile([P, CHW], F32, tag="ng")
                        nc.scalar.activation(out=ng[:, :w], in_=t2[:, :w],
                                             func=AF.Tanh)
                        zg = gp.tile([P, CHW], F32, tag="zg")
                        if first:
                            # h' = (1-z)*n ; 1-sigmoid(a) = sigmoid(-a)
                            nc.scalar.activation(out=zg[:, :w], in_=psz[:, :w],
                                                 func=AF.Sigmoid,
                                                 bias=bsum_s[:, 3:4], scale=-1.0)
                            nc.vector.tensor_mul(out=hT[:, c0:c1],
                                                 in0=zg[:, :w], in1=ng[:, :w])
                        else:
                            nc.scalar.activation(out=zg[:, :w], in_=psz[:, :w],
                                                 func=AF.Sigmoid,
                                                 bias=bsum_s[:, 1:2])
                            hm = gp.tile([P, CHW], F32, tag="hm")
                            nc.vector.tensor_sub(out=hm[:, :w],
                                                 in0=hT[:, c0:c1],
                                                 in1=ng[:, :w])
                            nc.vector.tensor_mul(out=hm[:, :w], in0=zg[:, :w],
                                                 in1=hm[:, :w])
                            nc.vector.tensor_add(out=hT[:, c0:c1],
                                                 in0=ng[:, :w], in1=hm[:, :w])

            # ---------------- output ----------------
            with (
                tc.tile_pool(name="oph", bufs=3) as op_,
                tc.tile_pool(name="opsum", bufs=2, space="PSUM") as ops,
            ):
                for i in range(NT):
                    pso = ops.tile([P, C], F32, tag="po")
                    nc.tensor.matmul(pso[:], lhsT=hT[:, i * P:(i + 1) * P],
                                     rhs=wo_s[:], start=True, stop=True)
                    og = op_.tile([P, C], F32, tag="og")
                    nc.vector.tensor_add(out=og[:], in0=pso[:], in1=bor_s[:])
                    nc.sync.dma_start(out=outT.ap()[i * P:(i + 1) * P, :],
                                      in_=og[:])

    nc.compile()
    return nc


def _o_code(n):
    """Interleaved U-table row index (in 64-f32 units) for node id n."""
    return (n & ~2047) | ((n & 127) << 4) | ((n >> 7) & 15)


def _prep(inputs):
    node_attr = np.asarray(inputs["node_attr"], np.float32)
    edge_index = np.asarray(inputs["edge_index"])
    slices = np.asarray(inputs["slices"])
    noise = np.asarray(inputs["noise"], np.float32)
    W1 = np.asarray(inputs["W1"], np.float32)
    b1 = np.asarray(inputs["b1"], np.float32)
    W2 = np.asarray(inputs["W2"], np.float32)
    Wx = np.asarray(inputs["Wx"], np.float32)
    Wh = np.asarray(inputs["Wh"], np.float32)
    bx = np.asarray(inputs["bx"], np.float32)
    bh = np.asarray(inputs["bh"], np.float32)
    Wo = np.asarray(inputs["Wo"], np.float32)
    bo = np.asarray(inputs["bo"], np.float32)

    # W2 sign permutation: positive-weight columns first; |W2| folded into W1.
    w2 = W2[:, 0]
    perm = np.concatenate([np.where(w2 > 0)[0], np.where(w2 <= 0)[0]])
    k_pos = int((w2 > 0).sum())
    w1s = (W1[:, perm] * np.abs(w2[perm])[None, :]).astype(np.float32)
    b1s = (b1[perm] * np.abs(w2[perm])).astype(np.float32)

    nat_pad = np.zeros((NPAD, C), np.float32)
    nat_pad[:N] = node_attr
    natT = np.ascontiguousarray(nat_pad.T)
    nat2 = nat_pad.reshape(NPAD // 2, 2 * C)

    # adjacency reordered per slices, then o-coded, in quad rows
    dst = edge_index[1].astype(np.int64)
    starts = slices[:, 0].astype(np.int64)
    dst2d = dst[starts[:, None] + np.arange(D)[None, :]]        # [N, D]
    quad = np.zeros((NPAD, D), np.int32)
    quad[:N] = _o_code(dst2d.astype(np.int32))
    quad = quad.reshape(NPAD // 4, 4 * D)

    w1u = np.ascontiguousarray(
        w1s[C:].reshape(T, C, HM).transpose(1, 0, 2).reshape(C, T * HM))
    w10 = np.ascontiguousarray(w1s[0:C])
    b1r = np.broadcast_to(b1s, (P, HM)).copy()
    bgx = np.ascontiguousarray(bx.reshape(3, HG).T)             # [128, 3]
    bgh = np.ascontiguousarray(bh.reshape(3, HG).T)
    bsum = np.zeros((P, 4), np.float32)
    bsum[:, 0:3] = bgx + bgh
    bsum[:, 3] = -bsum[:, 1]          # step-0 z-complement bias

    common = dict(
        natT=natT, nat2=nat2, quad=quad,
        w1u=w1u, w10=w10, b1r=b1r,
        wxd=np.ascontiguousarray(Wx), whd=np.ascontiguousarray(Wh),
        bgx=bgx, bgh=np.ascontiguousarray(bgh), bsum=bsum,
        wod=np.ascontiguousarray(Wo), bord=np.broadcast_to(bo, (P, C)).copy(),
    )

    in_maps = []
    for c in range(NCORES):
        ids = np.zeros(WPAD, np.int32)
        ids[:WPC] = np.arange(c * WPC, (c + 1) * WPC, dtype=np.int32)
        cur0 = np.ascontiguousarray(ids.reshape(NT, P).T)       # [P, NT]
        nz = np.zeros((T, WPAD, D), np.float32)
        nz[:, :WPC] = noise[:, c * WPC:(c + 1) * WPC]
        noiseR = np.ascontiguousarray(
            nz.reshape(T, NT, P, D).transpose(0, 2, 1, 3).reshape(T, P, NT * D))
        natTo = np.ascontiguousarray(nat_pad[c * WPC:c * WPC + WPAD].T)
        in_maps.append(dict(common, cur0=cur0, noiseR=noiseR, natTo=natTo))
    return in_maps, k_pos


def kernel(**inputs):
    global LAST_EXEC_NS, LAST_RESULTS
    in_maps, k_pos = _prep(inputs)
    if k_pos not in _CACHE:
        _CACHE[k_pos] = _build(k_pos)
    nc = _CACHE[k_pos]
    res = bass_utils.run_bass_kernel_spmd(nc, in_maps,
                                          core_ids=list(range(NCORES)),
                                          trace=TRACE)
    LAST_EXEC_NS = res.exec_time_ns
    LAST_RESULTS = res
    out = np.concatenate([res.results[c]["outT"][:WPC] for c in range(NCORES)])
    return out.astype(np.float32)
